# revision 11
# baseline (speedup 1.0000x reference)
"""Banded (sliding-window k=2) attention + residual + LayerNorm on 8 TRN2 cores.

Problem (per batch b): x (S=2048, D=1024)
  Q = x@Wq.T+bq ; K = x@Wk.T+bk ; V = x@Wv.T+bv
  scores = Q@K.T/sqrt(D), banded |i-j|<=k ; attn = softmax(scores)
  out = LN(attn@V + x) * gamma + beta
Returns (out (B,S,D), attn (B,S,S)).

Sharding: pure data-parallel over B — core b computes batch b. No collectives.

Per-core layout strategy (every matmul contracts over D, the contiguous axis
of both x and W, so both sides get PE-transposed on chip once):
  xT   [d,s]  <- PE transpose of x tiles (per 256-row chunk)
  WqT/WkT/WvT [d,e] <- PE transpose of weights (SBUF-resident)
  QT,KT [e,s] = WT.T @ xT ; V [s,e] = xT.T @ WvT (+bias on copy-out)
  score block per 128-row s-tile over window jw in [t*128-k, t*128+128+k):
      QT_tile.T @ KT[:, window] (+ additive band mask), softmax on-chip
  attn@V: PE-transpose attn-block pieces -> lhsT, rhs = V rows (+halo tiles)
  out = LN(attnV + x) via bn_stats/bn_aggr, * gamma + beta.
attn output: only band windows are written; the rest stays zero via the
pre-zeroed donated output buffers in run_bass_via_pjrt.
"""

from contextlib import ExitStack

import numpy as np

import concourse.bass as bass
import concourse.mybir as mybir
import concourse.tile as tile
from concourse import bacc, bass_utils

B, S, D = 8, 2048, 1024
N_CORES = 8
P = 128
CHUNK = 256                      # s rows per pipeline chunk (N for QT/KT matmuls)
N_CHUNKS = S // CHUNK            # 8
TPC = CHUNK // P                 # s-tiles per chunk = 2
N_TILES = S // P                 # 16
DT = D // P                      # d/e tiles = 8
LN_EPS = 1e-5
NEG = -1e30

MM_MODE = "f32r"                 # "f32r" (fast PE path) or "f32" (exact fp32)

_BUILD_CACHE: dict = {}


def _build(k: int, mm_mode: str):
    W_WIN = P + 2 * k            # score window width per 128-row s-tile
    f32 = mybir.dt.float32
    inv_scale = 1.0 / float(np.sqrt(D))

    def C(ap):
        if mm_mode == "f32r":
            return ap.bitcast(mybir.dt.float32r)
        return ap

    nc = bacc.Bacc(trn_type="TRN2", target_bir_lowering=False, debug=False,
                   num_devices=N_CORES, dynamic_dma_scratch_size=4096)

    x_d = nc.dram_tensor("x", [S, D], f32, kind="ExternalInput").ap()
    wq_d = nc.dram_tensor("Wq", [D, D], f32, kind="ExternalInput").ap()
    wk_d = nc.dram_tensor("Wk", [D, D], f32, kind="ExternalInput").ap()
    wv_d = nc.dram_tensor("Wv", [D, D], f32, kind="ExternalInput").ap()
    bq_d = nc.dram_tensor("bq", [D], f32, kind="ExternalInput").ap()
    bk_d = nc.dram_tensor("bk", [D], f32, kind="ExternalInput").ap()
    bv_d = nc.dram_tensor("bv", [D], f32, kind="ExternalInput").ap()
    gamma_d = nc.dram_tensor("gamma", [D], f32, kind="ExternalInput").ap()
    beta_d = nc.dram_tensor("beta", [D], f32, kind="ExternalInput").ap()
    out_d = nc.dram_tensor("out", [S, D], f32, kind="ExternalOutput").ap()
    attn_d = nc.dram_tensor("attn", [S, S], f32, kind="ExternalOutput").ap()

    with tile.TileContext(nc) as tc, ExitStack() as ctx:
        consts = ctx.enter_context(tc.tile_pool(name="consts", bufs=1))
        wt_pool = ctx.enter_context(tc.tile_pool(name="wt", bufs=1))
        x_pool = ctx.enter_context(tc.tile_pool(name="xp", bufs=2))
        xt_pool = ctx.enter_context(tc.tile_pool(name="xt", bufs=1))
        qt_pool = ctx.enter_context(tc.tile_pool(name="qt", bufs=2))
        kt_pool = ctx.enter_context(tc.tile_pool(name="ktp", bufs=2))
        v_pool = ctx.enter_context(tc.tile_pool(name="vp", bufs=2))
        tail_pool = ctx.enter_context(tc.tile_pool(name="tails", bufs=3))
        out_pool = ctx.enter_context(tc.tile_pool(name="outp", bufs=2))
        attn_pool = ctx.enter_context(tc.tile_pool(name="attnp", bufs=2))
        piece_pool = ctx.enter_context(tc.tile_pool(name="piece", bufs=2))
        small = ctx.enter_context(tc.tile_pool(name="small", bufs=4))
        ps = ctx.enter_context(tc.tile_pool(name="psp", bufs=8, space="PSUM"))

        def psum():
            return ps.tile([P, 512], f32, tag="ps", name="ps")

        # ---- constants ----------------------------------------------------
        ident = consts.tile([P, P], f32, tag="ident", name="ident")
        nc.gpsimd.memset(ident[:], 0.0)
        nc.gpsimd.affine_select(
            out=ident[:], in_=ident[:], compare_op=mybir.AluOpType.not_equal,
            fill=1.0, base=0, pattern=[[-1, P]], channel_multiplier=1)

        # additive band mask over window coords: valid iff 0 <= jw - i <= 2k
        maskI = consts.tile([P, W_WIN], f32, tag="maskI", name="maskI")
        nc.gpsimd.memset(maskI[:], 0.0)
        nc.gpsimd.affine_select(  # jw - i >= 0
            out=maskI[:], in_=maskI[:], compare_op=mybir.AluOpType.is_ge,
            fill=NEG, base=0, pattern=[[1, W_WIN]], channel_multiplier=-1)
        nc.gpsimd.affine_select(  # 2k - (jw - i) >= 0
            out=maskI[:], in_=maskI[:], compare_op=mybir.AluOpType.is_ge,
            fill=NEG, base=2 * k, pattern=[[-1, W_WIN]], channel_multiplier=1)

        # biases striped per e-tile: (p, m) = b[m*128+p]; bq pre-scaled
        bq_sc = consts.tile([P, DT], f32, tag="bq", name="bq_sc")
        nc.sync.dma_start(bq_sc[:], bq_d.rearrange("(o p) -> p o", p=P))
        nc.scalar.mul(bq_sc[:], bq_sc[:], inv_scale)
        bk_sb = consts.tile([P, DT], f32, tag="bk", name="bk_sb")
        nc.sync.dma_start(bk_sb[:], bk_d.rearrange("(o p) -> p o", p=P))

        def bcast_load(vec_ap, name):
            t = consts.tile([P, D], f32, tag=name, name=name)
            src = bass.AP(tensor=vec_ap.tensor, offset=vec_ap.offset,
                          ap=[[0, P]] + list(vec_ap.ap))
            nc.gpsimd.dma_start(out=t[:], in_=src)
            return t

        bv_bc = bcast_load(bv_d, "bv_bc")
        gamma_bc = bcast_load(gamma_d, "gamma_bc")
        beta_bc = bcast_load(beta_d, "beta_bc")

        eps_t = consts.tile([P, 1], f32, tag="eps", name="eps_t")
        nc.vector.memset(eps_t[:], LN_EPS)

        # ---- weight transposes: WT[p, dt, e] = W[e, dt*128+p] -------------
        def load_wt(w_d, name):
            wt = wt_pool.tile([P, DT, D], f32, tag=name, name=name)
            for et in range(DT):
                wnat = x_pool.tile([P, TPC, D], f32, tag="x", name="wnat")
                nc.sync.dma_start(wnat[:, 0, :], w_d[et * P:(et + 1) * P, :])
                for dt in range(DT):
                    tp = psum()
                    nc.tensor.transpose(tp[:, :P], wnat[:, 0, dt * P:(dt + 1) * P],
                                        ident[:])
                    if dt % 2 == 0:
                        nc.scalar.copy(out=C(wt[:, dt, et * P:(et + 1) * P]),
                                       in_=tp[:, :P])
                    else:
                        nc.vector.tensor_copy(out=C(wt[:, dt, et * P:(et + 1) * P]),
                                              in_=tp[:, :P])
            return wt

        wqT = load_wt(wq_d, "wqT")
        wkT = load_wt(wk_d, "wkT")
        wvT = load_wt(wv_d, "wvT")

        # ---- main pipeline ------------------------------------------------
        xs, xts, qts, kts, vs, kt_tails, v_tails = {}, {}, {}, {}, {}, {}, {}

        def b1(c):
            """Produce x/xT/QT/KT/V (+halo tails) for chunk c."""
            x_c = x_pool.tile([P, TPC, D], f32, tag="x", name="x_c")
            nc.sync.dma_start(
                x_c[:], x_d[c * CHUNK:(c + 1) * CHUNK, :]
                .rearrange("(u p) d -> p u d", p=P))
            xs[c] = x_c

            xt_c = xt_pool.tile([P, DT, CHUNK], f32, tag="xt", name="xt_c")
            for u in range(TPC):
                for dt in range(DT):
                    tp = psum()
                    nc.tensor.transpose(tp[:, :P], x_c[:, u, dt * P:(dt + 1) * P],
                                        ident[:])
                    nc.vector.tensor_copy(out=C(xt_c[:, dt, u * P:(u + 1) * P]),
                                          in_=tp[:, :P])
            xts[c] = xt_c

            qt_c = qt_pool.tile([P, DT, CHUNK], f32, tag="qt", name="qt_c")
            kt_c = kt_pool.tile([P, DT, CHUNK], f32, tag="kt", name="kt_c")
            for wt, dst, bias, scale in (
                    (wqT, qt_c, bq_sc, inv_scale),
                    (wkT, kt_c, bk_sb, 1.0)):
                for m in range(DT):
                    acc = psum()
                    for kk in range(DT):
                        nc.tensor.matmul(
                            acc[:, :CHUNK],
                            C(wt[:, kk, m * P:(m + 1) * P]),
                            C(xt_c[:, kk, :]),
                            start=(kk == 0), stop=(kk == DT - 1))
                    nc.scalar.activation(
                        C(dst[:, m, :]), acc[:, :CHUNK],
                        mybir.ActivationFunctionType.Identity,
                        bias=bias[:, m:m + 1], scale=scale)
            qts[c], kts[c] = qt_c, kt_c

            ktt = tail_pool.tile([P, DT, k], f32, tag="ktt", name="ktt")
            nc.vector.tensor_copy(out=C(ktt[:]),
                                  in_=C(kt_c[:, :, CHUNK - k:CHUNK]))
            kt_tails[c] = ktt

            v_c = v_pool.tile([P, TPC, D], f32, tag="v", name="v_c")
            for u in range(TPC):
                for nch in range(D // 512):
                    acc = psum()
                    for kk in range(DT):
                        nc.tensor.matmul(
                            acc[:],
                            C(xt_c[:, kk, u * P:(u + 1) * P]),
                            C(wvT[:, kk, nch * 512:(nch + 1) * 512]),
                            start=(kk == 0), stop=(kk == DT - 1))
                    nc.vector.tensor_tensor(
                        out=C(v_c[:, u, nch * 512:(nch + 1) * 512]), in0=acc[:],
                        in1=bv_bc[:, nch * 512:(nch + 1) * 512],
                        op=mybir.AluOpType.add)
            vs[c] = v_c

            # halo: last k V rows of each s-tile, re-based to partition 0
            # (matmul operands must start at partition 0/32/64) — packed in
            # the free dim: [:, 0:D] = tail of tile u=0, [:, D:2D] = u=1.
            vt = tail_pool.tile([k, 2 * D], f32, tag="vt", name="vt")
            nc.sync.dma_start(C(vt[:, 0:D]), C(v_c[P - k:P, 0, :]))
            nc.sync.dma_start(C(vt[:, D:2 * D]), C(v_c[P - k:P, 1, :]))
            v_tails[c] = vt

        def b2(c):
            """Scores, softmax, attn write, attn@V, residual+LN for chunk c."""
            qt_c, kt_c, v_c, x_c = qts[c], kts[c], vs[c], xs[c]
            for u in range(TPC):
                t = c * TPC + u
                lo = k if t == 0 else 0
                hi = P + k if t == N_TILES - 1 else W_WIN

                # main band block (own-chunk KT) and k-wide halo block
                # (neighbor-chunk KT) accumulate in separate PSUM tiles —
                # one accumulation group per PSUM region.
                qsl = slice(0, P) if u == 0 else slice(P, CHUNK)
                if u == 0:
                    main_sl = slice(k, W_WIN)
                    edge_sl = slice(0, k) if t > 0 else None
                else:
                    main_sl = slice(0, P + k)
                    edge_sl = slice(P + k, W_WIN) if t < N_TILES - 1 else None
                s_ps = psum()
                e_ps = psum() if edge_sl is not None else None
                for kk in range(DT):
                    nc.tensor.matmul(
                        s_ps[:, main_sl],
                        C(qt_c[:, kk, qsl]),
                        C(kt_c[:, kk, 0:P + k] if u == 0
                          else kt_c[:, kk, P - k:CHUNK]),
                        start=(kk == 0), stop=(kk == DT - 1))
                if edge_sl is not None:
                    for kk in range(DT):
                        nc.tensor.matmul(
                            e_ps[:, edge_sl],
                            C(qt_c[:, kk, qsl]),
                            C(kt_tails[c - 1][:, kk, :] if u == 0
                              else kts[c + 1][:, kk, 0:k]),
                            start=(kk == 0), stop=(kk == DT - 1))

                attn_sb = attn_pool.tile([P, W_WIN], f32, tag="attn",
                                         name="attn_sb")
                nc.vector.tensor_tensor(out=attn_sb[:, main_sl],
                                        in0=s_ps[:, main_sl],
                                        in1=maskI[:, main_sl],
                                        op=mybir.AluOpType.add)
                if edge_sl is not None:
                    nc.vector.tensor_tensor(out=attn_sb[:, edge_sl],
                                            in0=e_ps[:, edge_sl],
                                            in1=maskI[:, edge_sl],
                                            op=mybir.AluOpType.add)
                negmax = small.tile([P, 1], f32, tag="negmax", name="negmax")
                nc.vector.tensor_reduce(out=negmax[:], in_=attn_sb[:, lo:hi],
                                        axis=mybir.AxisListType.X,
                                        op=mybir.AluOpType.max, negate=True)
                rowsum = small.tile([P, 1], f32, tag="rowsum", name="rowsum")
                nc.scalar.activation(attn_sb[:, lo:hi], attn_sb[:, lo:hi],
                                     mybir.ActivationFunctionType.Exp,
                                     bias=negmax[:], scale=1.0,
                                     accum_out=rowsum[:])
                rinv = small.tile([P, 1], f32, tag="rinv", name="rinv")
                nc.vector.reciprocal(rinv[:], rowsum[:])
                nc.vector.tensor_scalar_mul(attn_sb[:, lo:hi], attn_sb[:, lo:hi],
                                            rinv[:])
                nc.sync.dma_start(
                    attn_d[t * P:(t + 1) * P, t * P - k + lo:t * P - k + hi],
                    attn_sb[:, lo:hi])

                # transposed attn pieces (lhsT for attn @ V)
                pc_mid = piece_pool.tile([P, P], f32, tag="pmid", name="pc_mid")
                tp = psum()
                nc.tensor.transpose(tp[:, :P], attn_sb[:, k:P + k], ident[:])
                nc.vector.tensor_copy(out=C(pc_mid[:]), in_=tp[:, :P])
                pc_left = pc_right = None
                if t > 0:
                    pc_left = piece_pool.tile([k, P], f32, tag="pleft",
                                              name="pc_left")
                    tp = psum()
                    nc.tensor.transpose(tp[:k, :P], attn_sb[:, 0:k], ident[:])
                    nc.vector.tensor_copy(out=C(pc_left[:]), in_=tp[:k, :P])
                if t < N_TILES - 1:
                    pc_right = piece_pool.tile([k, P], f32, tag="pright",
                                               name="pc_right")
                    tp = psum()
                    nc.tensor.transpose(tp[:k, :P], attn_sb[:, P + k:W_WIN],
                                        ident[:])
                    nc.vector.tensor_copy(out=C(pc_right[:]), in_=tp[:k, :P])

                out_sb = out_pool.tile([P, D], f32, tag="out", name="out_sb")
                for nch in range(D // 512):
                    sl = slice(nch * 512, (nch + 1) * 512)
                    o_ps = psum()
                    mms = [(pc_mid[:], v_c[:, u, sl])]
                    if pc_left is not None:
                        left_rhs = (
                            v_tails[c - 1][:, D + nch * 512:D + (nch + 1) * 512]
                            if u == 0 else v_tails[c][:, sl])
                        mms.append((pc_left[:], left_rhs))
                    if pc_right is not None:
                        right_rhs = (v_c[0:k, 1, sl] if u == 0
                                     else vs[c + 1][0:k, 0, sl])
                        mms.append((pc_right[:], right_rhs))
                    for i, (lhsT, rhs) in enumerate(mms):
                        nc.tensor.matmul(o_ps[:], C(lhsT), C(rhs),
                                         start=(i == 0),
                                         stop=(i == len(mms) - 1))
                    nc.vector.tensor_tensor(out=out_sb[:, sl], in0=o_ps[:],
                                            in1=x_c[:, u, sl],
                                            op=mybir.AluOpType.add)

                # LayerNorm over D (free dim)
                stats = small.tile([P, 2, 6], f32, tag="stats", name="stats")
                nc.vector.bn_stats(stats[:, 0, :], out_sb[:, 0:512])
                nc.vector.bn_stats(stats[:, 1, :], out_sb[:, 512:1024])
                mv = small.tile([P, 2], f32, tag="mv", name="mv")
                nc.vector.bn_aggr(mv[:], stats[:])
                rstd = small.tile([P, 1], f32, tag="rstd", name="rstd")
                nc.scalar.activation(rstd[:], mv[:, 1:2],
                                     mybir.ActivationFunctionType.Sqrt,
                                     bias=eps_t[:])
                nc.vector.reciprocal(rstd[:], rstd[:])
                nc.vector.tensor_scalar(out_sb[:], out_sb[:],
                                        scalar1=mv[:, 0:1], scalar2=rstd[:],
                                        op0=mybir.AluOpType.subtract,
                                        op1=mybir.AluOpType.mult)
                nc.vector.tensor_tensor(out=out_sb[:], in0=out_sb[:],
                                        in1=gamma_bc[:],
                                        op=mybir.AluOpType.mult)
                nc.vector.tensor_tensor(out=out_sb[:], in0=out_sb[:],
                                        in1=beta_bc[:], op=mybir.AluOpType.add)
                nc.sync.dma_start(out_d[t * P:(t + 1) * P, :], out_sb[:])

        b1(0)
        for c in range(1, N_CHUNKS):
            b1(c)
            b2(c - 1)
        b2(N_CHUNKS - 1)

    nc.compile()
    return nc


def _get_nc(k: int):
    key = (k, MM_MODE)
    if key not in _BUILD_CACHE:
        _BUILD_CACHE[key] = _build(k, MM_MODE)
    return _BUILD_CACHE[key]


def run(x, Wq, bq, Wk, bk, Wv, bv, gamma, beta, k, trace=False):
    k = int(k)
    assert 1 <= k <= 32
    nc = _get_nc(k)
    x = np.ascontiguousarray(np.asarray(x, dtype=np.float32))
    in_common = {
        "Wq": np.ascontiguousarray(np.asarray(Wq, np.float32)),
        "Wk": np.ascontiguousarray(np.asarray(Wk, np.float32)),
        "Wv": np.ascontiguousarray(np.asarray(Wv, np.float32)),
        "bq": np.ascontiguousarray(np.asarray(bq, np.float32)),
        "bk": np.ascontiguousarray(np.asarray(bk, np.float32)),
        "bv": np.ascontiguousarray(np.asarray(bv, np.float32)),
        "gamma": np.ascontiguousarray(np.asarray(gamma, np.float32)),
        "beta": np.ascontiguousarray(np.asarray(beta, np.float32)),
    }
    in_maps = [dict(in_common, x=x[b]) for b in range(B)]
    res = bass_utils.run_bass_kernel_spmd(
        nc, in_maps, core_ids=list(range(N_CORES)), trace=trace)
    out = np.stack([res.results[b]["out"] for b in range(B)])
    attn = np.stack([res.results[b]["attn"] for b in range(B)])
    return (out, attn), res


def kernel(x, Wq, bq, Wk, bk, Wv, bv, gamma, beta, k):
    (out, attn), _ = run(x, Wq, bq, Wk, bk, Wv, bv, gamma, beta, k)
    return out, attn


# revision 14
# speedup vs baseline: 1.1397x; 1.1397x over previous
"""Banded (sliding-window k=2) attention + residual + LayerNorm on 8 TRN2 cores.

Problem (per batch b): x (S=2048, D=1024)
  Q = x@Wq.T+bq ; K = x@Wk.T+bk ; V = x@Wv.T+bv
  scores = Q@K.T/sqrt(D), banded |i-j|<=k ; attn = softmax(scores)
  out = LN(attn@V + x) * gamma + beta
Returns (out (B,S,D), attn (B,S,S)).

Sharding: pure data-parallel over B — core b computes batch b. No collectives.

Per-core layout strategy (every matmul contracts over D, the contiguous axis
of both x and W, so both sides get PE-transposed on chip once):
  xT   [d,s]  <- PE transpose of x tiles (per 256-row chunk)
  WqT/WkT/WvT [d,e] <- PE transpose of weights (SBUF-resident)
  QT,KT [e,s] = WT.T @ xT ; V [s,e] = xT.T @ WvT (+bias on copy-out)
  score block per 128-row s-tile over window jw in [t*128-k, t*128+128+k):
      QT_tile.T @ KT[:, window] (+ additive band mask), softmax on-chip
  attn@V: PE-transpose attn-block pieces -> lhsT, rhs = V rows (+halo tiles)
  out = LN(attnV + x) via bn_stats/bn_aggr, * gamma + beta.
attn output: only band windows are written; the rest stays zero via the
pre-zeroed donated output buffers in run_bass_via_pjrt.
"""

from contextlib import ExitStack

import numpy as np

import concourse.bass as bass
import concourse.mybir as mybir
import concourse.tile as tile
from concourse import bacc, bass_utils

B, S, D = 8, 2048, 1024
N_CORES = 8
P = 128
CHUNK = 256                      # s rows per pipeline chunk (N for QT/KT matmuls)
N_CHUNKS = S // CHUNK            # 8
TPC = CHUNK // P                 # s-tiles per chunk = 2
N_TILES = S // P                 # 16
DT = D // P                      # d/e tiles = 8
LN_EPS = 1e-5
NEG = -1e30

MM_MODE = "f32r"                 # "f32r" (fast PE path) or "f32" (exact fp32)

_BUILD_CACHE: dict = {}


def _build(k: int, mm_mode: str):
    W_WIN = P + 2 * k            # score window width per 128-row s-tile
    f32 = mybir.dt.float32
    inv_scale = 1.0 / float(np.sqrt(D))

    def C(ap):
        if mm_mode == "f32r":
            return ap.bitcast(mybir.dt.float32r)
        return ap

    nc = bacc.Bacc(trn_type="TRN2", target_bir_lowering=False, debug=False,
                   num_devices=N_CORES, dynamic_dma_scratch_size=4096)

    x_d = nc.dram_tensor("x", [S, D], f32, kind="ExternalInput").ap()
    wq_d = nc.dram_tensor("Wq", [D, D], f32, kind="ExternalInput").ap()
    wk_d = nc.dram_tensor("Wk", [D, D], f32, kind="ExternalInput").ap()
    wv_d = nc.dram_tensor("Wv", [D, D], f32, kind="ExternalInput").ap()
    bq_d = nc.dram_tensor("bq", [D], f32, kind="ExternalInput").ap()
    bk_d = nc.dram_tensor("bk", [D], f32, kind="ExternalInput").ap()
    bv_d = nc.dram_tensor("bv", [D], f32, kind="ExternalInput").ap()
    gamma_d = nc.dram_tensor("gamma", [D], f32, kind="ExternalInput").ap()
    beta_d = nc.dram_tensor("beta", [D], f32, kind="ExternalInput").ap()
    out_d = nc.dram_tensor("out", [S, D], f32, kind="ExternalOutput").ap()
    attn_d = nc.dram_tensor("attn", [S, S], f32, kind="ExternalOutput").ap()

    with tile.TileContext(nc) as tc, ExitStack() as ctx:
        consts = ctx.enter_context(tc.tile_pool(name="consts", bufs=1))
        wt_pool = ctx.enter_context(tc.tile_pool(name="wt", bufs=1))
        x_pool = ctx.enter_context(tc.tile_pool(name="xp", bufs=2))
        xt_pool = ctx.enter_context(tc.tile_pool(name="xt", bufs=1))
        qt_pool = ctx.enter_context(tc.tile_pool(name="qt", bufs=2))
        kt_pool = ctx.enter_context(tc.tile_pool(name="ktp", bufs=2))
        v_pool = ctx.enter_context(tc.tile_pool(name="vp", bufs=2))
        tail_pool = ctx.enter_context(tc.tile_pool(name="tails", bufs=3))
        out_pool = ctx.enter_context(tc.tile_pool(name="outp", bufs=2))
        attn_pool = ctx.enter_context(tc.tile_pool(name="attnp", bufs=3))
        piece_pool = ctx.enter_context(tc.tile_pool(name="piece", bufs=2))
        small = ctx.enter_context(tc.tile_pool(name="small", bufs=4))
        ps = ctx.enter_context(tc.tile_pool(name="psp", bufs=8, space="PSUM"))

        def psum():
            return ps.tile([P, 512], f32, tag="ps", name="ps")

        # ---- constants ----------------------------------------------------
        ident = consts.tile([P, P], f32, tag="ident", name="ident")
        nc.gpsimd.memset(ident[:], 0.0)
        nc.gpsimd.affine_select(
            out=ident[:], in_=ident[:], compare_op=mybir.AluOpType.not_equal,
            fill=1.0, base=0, pattern=[[-1, P]], channel_multiplier=1)

        # additive band mask over window coords: valid iff 0 <= jw - i <= 2k
        maskI = consts.tile([P, W_WIN], f32, tag="maskI", name="maskI")
        nc.gpsimd.memset(maskI[:], 0.0)
        nc.gpsimd.affine_select(  # jw - i >= 0
            out=maskI[:], in_=maskI[:], compare_op=mybir.AluOpType.is_ge,
            fill=NEG, base=0, pattern=[[1, W_WIN]], channel_multiplier=-1)
        nc.gpsimd.affine_select(  # 2k - (jw - i) >= 0
            out=maskI[:], in_=maskI[:], compare_op=mybir.AluOpType.is_ge,
            fill=NEG, base=2 * k, pattern=[[-1, W_WIN]], channel_multiplier=1)

        # biases striped per e-tile: (p, m) = b[m*128+p]; bq pre-scaled
        bq_sc = consts.tile([P, DT], f32, tag="bq", name="bq_sc")
        nc.sync.dma_start(bq_sc[:], bq_d.rearrange("(o p) -> p o", p=P))
        nc.scalar.mul(bq_sc[:], bq_sc[:], inv_scale)
        bk_sb = consts.tile([P, DT], f32, tag="bk", name="bk_sb")
        nc.sync.dma_start(bk_sb[:], bk_d.rearrange("(o p) -> p o", p=P))

        def bcast_load(vec_ap, name):
            t = consts.tile([P, D], f32, tag=name, name=name)
            src = bass.AP(tensor=vec_ap.tensor, offset=vec_ap.offset,
                          ap=[[0, P]] + list(vec_ap.ap))
            nc.gpsimd.dma_start(out=t[:], in_=src)
            return t

        bv_bc = bcast_load(bv_d, "bv_bc")
        gamma_bc = bcast_load(gamma_d, "gamma_bc")
        beta_bc = bcast_load(beta_d, "beta_bc")

        eps_t = consts.tile([P, 1], f32, tag="eps", name="eps_t")
        nc.vector.memset(eps_t[:], LN_EPS)

        # ---- weight transposes: WT[p, dt, e] = W[e, dt*128+p] -------------
        def load_wt(w_d, name):
            wt = wt_pool.tile([P, DT, D], f32, tag=name, name=name)
            for et in range(DT):
                wnat = x_pool.tile([P, TPC, D], f32, tag="x", name="wnat")
                nc.sync.dma_start(wnat[:, 0, :], w_d[et * P:(et + 1) * P, :])
                for dt in range(DT):
                    tp = psum()
                    nc.tensor.transpose(tp[:, :P], wnat[:, 0, dt * P:(dt + 1) * P],
                                        ident[:])
                    if dt % 2 == 0:
                        nc.scalar.copy(out=C(wt[:, dt, et * P:(et + 1) * P]),
                                       in_=tp[:, :P])
                    else:
                        nc.vector.tensor_copy(out=C(wt[:, dt, et * P:(et + 1) * P]),
                                              in_=tp[:, :P])
            return wt

        wqT = load_wt(wq_d, "wqT")
        wkT = load_wt(wk_d, "wkT")
        wvT = load_wt(wv_d, "wvT")

        # ---- main pipeline ------------------------------------------------
        xs, xts, qts, kts, vs, v_tails = {}, {}, {}, {}, {}, {}

        def b1(c):
            """Produce x/xT/QT/KT/V (+halo tails) for chunk c."""
            x_c = x_pool.tile([P, TPC, D], f32, tag="x", name="x_c")
            nc.sync.dma_start(
                x_c[:], x_d[c * CHUNK:(c + 1) * CHUNK, :]
                .rearrange("(u p) d -> p u d", p=P))
            xs[c] = x_c

            xt_c = xt_pool.tile([P, DT, CHUNK], f32, tag="xt", name="xt_c")
            for u in range(TPC):
                for dt in range(DT):
                    tp = psum()
                    nc.tensor.transpose(tp[:, :P], x_c[:, u, dt * P:(dt + 1) * P],
                                        ident[:])
                    nc.vector.tensor_copy(out=C(xt_c[:, dt, u * P:(u + 1) * P]),
                                          in_=tp[:, :P])
            xts[c] = xt_c

            qt_c = qt_pool.tile([P, DT, CHUNK], f32, tag="qt", name="qt_c")
            # KT gets k halo columns on both sides so each score block is a
            # single contiguous rhs window: col h+j holds KT[:, c*CHUNK+j].
            kt_c = kt_pool.tile([P, DT, CHUNK + 2 * k], f32, tag="kt",
                                name="kt_c")
            if c == 0:  # never-written halos must still be finite
                nc.vector.memset(kt_c[:, :, 0:k], 0.0)
            if c == N_CHUNKS - 1:
                nc.vector.memset(kt_c[:, :, k + CHUNK:], 0.0)
            for wt, dst, off, bias, scale in (
                    (wqT, qt_c, 0, bq_sc, inv_scale),
                    (wkT, kt_c, k, bk_sb, 1.0)):
                for m in range(DT):
                    acc = psum()
                    for kk in range(DT):
                        nc.tensor.matmul(
                            acc[:, :CHUNK],
                            C(wt[:, kk, m * P:(m + 1) * P]),
                            C(xt_c[:, kk, :]),
                            start=(kk == 0), stop=(kk == DT - 1))
                    nc.scalar.activation(
                        C(dst[:, m, off:off + CHUNK]), acc[:, :CHUNK],
                        mybir.ActivationFunctionType.Identity,
                        bias=bias[:, m:m + 1], scale=scale)
            qts[c], kts[c] = qt_c, kt_c

            if c > 0:  # exchange halo columns with the previous chunk
                nc.vector.tensor_copy(
                    out=C(kts[c - 1][:, :, k + CHUNK:]),
                    in_=C(kt_c[:, :, k:2 * k]))
                nc.vector.tensor_copy(
                    out=C(kt_c[:, :, 0:k]),
                    in_=C(kts[c - 1][:, :, CHUNK:CHUNK + k]))

            v_c = v_pool.tile([P, TPC, D], f32, tag="v", name="v_c")
            for u in range(TPC):
                for nch in range(D // 512):
                    acc = psum()
                    for kk in range(DT):
                        nc.tensor.matmul(
                            acc[:],
                            C(xt_c[:, kk, u * P:(u + 1) * P]),
                            C(wvT[:, kk, nch * 512:(nch + 1) * 512]),
                            start=(kk == 0), stop=(kk == DT - 1))
                    nc.vector.tensor_tensor(
                        out=C(v_c[:, u, nch * 512:(nch + 1) * 512]), in0=acc[:],
                        in1=bv_bc[:, nch * 512:(nch + 1) * 512],
                        op=mybir.AluOpType.add)
            vs[c] = v_c

            # halo: last k V rows of each s-tile, re-based to partition 0
            # (matmul operands must start at partition 0/32/64) — packed in
            # the free dim: [:, 0:D] = tail of tile u=0, [:, D:2D] = u=1.
            vt = tail_pool.tile([k, 2 * D], f32, tag="vt", name="vt")
            nc.sync.dma_start(C(vt[:, 0:D]), C(v_c[P - k:P, 0, :]))
            nc.sync.dma_start(C(vt[:, D:2 * D]), C(v_c[P - k:P, 1, :]))
            v_tails[c] = vt

        def b2(c):
            """Scores, softmax, attn write, attn@V, residual+LN for chunk c.

            Phase-ordered: softmax for both s-tiles first, then attnV+LN for
            both — keeps ACT on one activation table (Exp..Exp, Sqrt..Sqrt)
            and overlaps tile 1's softmax with tile 0's attn@V on PE.
            """
            qt_c, kt_c, v_c, x_c = qts[c], kts[c], vs[c], xs[c]
            attns = {}
            for u in range(TPC):
                t = c * TPC + u
                lo = k if t == 0 else 0
                hi = P + k if t == N_TILES - 1 else W_WIN

                # one contiguous rhs window thanks to the KT halo columns
                s_ps = psum()
                for kk in range(DT):
                    nc.tensor.matmul(
                        s_ps[:, 0:W_WIN],
                        C(qt_c[:, kk, u * P:(u + 1) * P]),
                        C(kt_c[:, kk, u * P:u * P + W_WIN]),
                        start=(kk == 0), stop=(kk == DT - 1))

                attn_sb = attn_pool.tile([P, W_WIN], f32, tag="attn",
                                         name="attn_sb")
                nc.vector.tensor_tensor(out=attn_sb[:, lo:hi],
                                        in0=s_ps[:, lo:hi],
                                        in1=maskI[:, lo:hi],
                                        op=mybir.AluOpType.add)
                negmax = small.tile([P, 1], f32, tag="negmax", name="negmax")
                nc.vector.tensor_reduce(out=negmax[:], in_=attn_sb[:, lo:hi],
                                        axis=mybir.AxisListType.X,
                                        op=mybir.AluOpType.max, negate=True)
                rowsum = small.tile([P, 1], f32, tag="rowsum", name="rowsum")
                nc.scalar.activation(attn_sb[:, lo:hi], attn_sb[:, lo:hi],
                                     mybir.ActivationFunctionType.Exp,
                                     bias=negmax[:], scale=1.0,
                                     accum_out=rowsum[:])
                rinv = small.tile([P, 1], f32, tag="rinv", name="rinv")
                nc.vector.reciprocal(rinv[:], rowsum[:])
                nc.vector.tensor_scalar_mul(attn_sb[:, lo:hi],
                                            attn_sb[:, lo:hi], rinv[:])
                nc.sync.dma_start(
                    attn_d[t * P:(t + 1) * P, t * P - k + lo:t * P - k + hi],
                    attn_sb[:, lo:hi])
                attns[u] = attn_sb

            for u in range(TPC):
                t = c * TPC + u
                attn_sb = attns[u]
                # transposed attn pieces (lhsT for attn @ V) — all three share
                # one PSUM tile in disjoint column regions.
                tp = psum()
                pc_mid = piece_pool.tile([P, P], f32, tag="pmid", name="pc_mid")
                nc.tensor.transpose(tp[:, 0:P], attn_sb[:, k:P + k], ident[:])
                nc.vector.tensor_copy(out=C(pc_mid[:]), in_=tp[:, 0:P])
                pc_left = pc_right = None
                if t > 0:
                    pc_left = piece_pool.tile([k, P], f32, tag="pleft",
                                              name="pc_left")
                    nc.tensor.transpose(tp[:k, P:2 * P], attn_sb[:, 0:k],
                                        ident[:])
                    nc.vector.tensor_copy(out=C(pc_left[:]), in_=tp[:k, P:2 * P])
                if t < N_TILES - 1:
                    pc_right = piece_pool.tile([k, P], f32, tag="pright",
                                               name="pc_right")
                    nc.tensor.transpose(tp[:k, 2 * P:3 * P],
                                        attn_sb[:, P + k:W_WIN], ident[:])
                    nc.vector.tensor_copy(out=C(pc_right[:]),
                                          in_=tp[:k, 2 * P:3 * P])

                out_sb = out_pool.tile([P, D], f32, tag="out", name="out_sb")
                for nch in range(D // 512):
                    sl = slice(nch * 512, (nch + 1) * 512)
                    o_ps = psum()
                    mms = [(pc_mid[:], v_c[:, u, sl])]
                    if pc_left is not None:
                        left_rhs = (
                            v_tails[c - 1][:, D + nch * 512:D + (nch + 1) * 512]
                            if u == 0 else v_tails[c][:, sl])
                        mms.append((pc_left[:], left_rhs))
                    if pc_right is not None:
                        right_rhs = (v_c[0:k, 1, sl] if u == 0
                                     else vs[c + 1][0:k, 0, sl])
                        mms.append((pc_right[:], right_rhs))
                    for i, (lhsT, rhs) in enumerate(mms):
                        nc.tensor.matmul(o_ps[:], C(lhsT), C(rhs),
                                         start=(i == 0),
                                         stop=(i == len(mms) - 1))
                    nc.vector.tensor_tensor(out=out_sb[:, sl], in0=o_ps[:],
                                            in1=x_c[:, u, sl],
                                            op=mybir.AluOpType.add)

                # LayerNorm over D (free dim)
                stats = small.tile([P, 2, 6], f32, tag="stats", name="stats")
                nc.vector.bn_stats(stats[:, 0, :], out_sb[:, 0:512])
                nc.vector.bn_stats(stats[:, 1, :], out_sb[:, 512:1024])
                mv = small.tile([P, 2], f32, tag="mv", name="mv")
                nc.vector.bn_aggr(mv[:], stats[:])
                rstd = small.tile([P, 1], f32, tag="rstd", name="rstd")
                nc.scalar.activation(rstd[:], mv[:, 1:2],
                                     mybir.ActivationFunctionType.Sqrt,
                                     bias=eps_t[:])
                nc.vector.reciprocal(rstd[:], rstd[:])
                nc.vector.tensor_scalar(out_sb[:], out_sb[:],
                                        scalar1=mv[:, 0:1], scalar2=rstd[:],
                                        op0=mybir.AluOpType.subtract,
                                        op1=mybir.AluOpType.mult)
                nc.vector.tensor_tensor(out=out_sb[:], in0=out_sb[:],
                                        in1=gamma_bc[:],
                                        op=mybir.AluOpType.mult)
                nc.vector.tensor_tensor(out=out_sb[:], in0=out_sb[:],
                                        in1=beta_bc[:], op=mybir.AluOpType.add)
                nc.sync.dma_start(out_d[t * P:(t + 1) * P, :], out_sb[:])

        b1(0)
        for c in range(1, N_CHUNKS):
            b1(c)
            b2(c - 1)
        b2(N_CHUNKS - 1)

    nc.compile()
    return nc


def _get_nc(k: int):
    key = (k, MM_MODE)
    if key not in _BUILD_CACHE:
        _BUILD_CACHE[key] = _build(k, MM_MODE)
    return _BUILD_CACHE[key]


def run(x, Wq, bq, Wk, bk, Wv, bv, gamma, beta, k, trace=False):
    k = int(k)
    assert 1 <= k <= 32
    nc = _get_nc(k)
    x = np.ascontiguousarray(np.asarray(x, dtype=np.float32))
    in_common = {
        "Wq": np.ascontiguousarray(np.asarray(Wq, np.float32)),
        "Wk": np.ascontiguousarray(np.asarray(Wk, np.float32)),
        "Wv": np.ascontiguousarray(np.asarray(Wv, np.float32)),
        "bq": np.ascontiguousarray(np.asarray(bq, np.float32)),
        "bk": np.ascontiguousarray(np.asarray(bk, np.float32)),
        "bv": np.ascontiguousarray(np.asarray(bv, np.float32)),
        "gamma": np.ascontiguousarray(np.asarray(gamma, np.float32)),
        "beta": np.ascontiguousarray(np.asarray(beta, np.float32)),
    }
    in_maps = [dict(in_common, x=x[b]) for b in range(B)]
    res = bass_utils.run_bass_kernel_spmd(
        nc, in_maps, core_ids=list(range(N_CORES)), trace=trace)
    out = np.stack([res.results[b]["out"] for b in range(B)])
    attn = np.stack([res.results[b]["attn"] for b in range(B)])
    return (out, attn), res


def kernel(x, Wq, bq, Wk, bk, Wv, bv, gamma, beta, k):
    (out, attn), _ = run(x, Wq, bq, Wk, bk, Wv, bv, gamma, beta, k)
    return out, attn


# revision 15
# speedup vs baseline: 1.1677x; 1.0246x over previous
"""Banded (sliding-window k=2) attention + residual + LayerNorm on 8 TRN2 cores.

Problem (per batch b): x (S=2048, D=1024)
  Q = x@Wq.T+bq ; K = x@Wk.T+bk ; V = x@Wv.T+bv
  scores = Q@K.T/sqrt(D), banded |i-j|<=k ; attn = softmax(scores)
  out = LN(attn@V + x) * gamma + beta
Returns (out (B,S,D), attn (B,S,S)).

Sharding: pure data-parallel over B — core b computes batch b. No collectives.

Per-core layout strategy (every matmul contracts over D, the contiguous axis
of both x and W, so both sides get PE-transposed on chip once):
  xT   [d,s]  <- PE transpose of x tiles (per 256-row chunk)
  WqT/WkT/WvT [d,e] <- PE transpose of weights (SBUF-resident)
  QT,KT [e,s] = WT.T @ xT ; V [s,e] = xT.T @ WvT (+bias on copy-out)
  score block per 128-row s-tile over window jw in [t*128-k, t*128+128+k):
      QT_tile.T @ KT[:, window] (+ additive band mask), softmax on-chip
  attn@V: PE-transpose attn-block pieces -> lhsT, rhs = V rows (+halo tiles)
  out = LN(attnV + x) via bn_stats/bn_aggr, * gamma + beta.
attn output: only band windows are written; the rest stays zero via the
pre-zeroed donated output buffers in run_bass_via_pjrt.
"""

from contextlib import ExitStack

import numpy as np

import concourse.bass as bass
import concourse.mybir as mybir
import concourse.tile as tile
from concourse import bacc, bass_utils

B, S, D = 8, 2048, 1024
N_CORES = 8
P = 128
CHUNK = 256                      # s rows per pipeline chunk (N for QT/KT matmuls)
N_CHUNKS = S // CHUNK            # 8
TPC = CHUNK // P                 # s-tiles per chunk = 2
N_TILES = S // P                 # 16
DT = D // P                      # d/e tiles = 8
LN_EPS = 1e-5
NEG = -1e30

MM_MODE = "f32r"                 # "f32r" (fast PE path) or "f32" (exact fp32)

_BUILD_CACHE: dict = {}


def _build(k: int, mm_mode: str, apply_gb: bool = True):
    W_WIN = P + 2 * k            # score window width per 128-row s-tile
    f32 = mybir.dt.float32
    inv_scale = 1.0 / float(np.sqrt(D))

    def C(ap):
        if mm_mode == "f32r":
            return ap.bitcast(mybir.dt.float32r)
        return ap

    nc = bacc.Bacc(trn_type="TRN2", target_bir_lowering=False, debug=False,
                   num_devices=N_CORES, dynamic_dma_scratch_size=4096)

    x_d = nc.dram_tensor("x", [S, D], f32, kind="ExternalInput").ap()
    wq_d = nc.dram_tensor("Wq", [D, D], f32, kind="ExternalInput").ap()
    wk_d = nc.dram_tensor("Wk", [D, D], f32, kind="ExternalInput").ap()
    wv_d = nc.dram_tensor("Wv", [D, D], f32, kind="ExternalInput").ap()
    bq_d = nc.dram_tensor("bq", [D], f32, kind="ExternalInput").ap()
    bk_d = nc.dram_tensor("bk", [D], f32, kind="ExternalInput").ap()
    bv_d = nc.dram_tensor("bv", [D], f32, kind="ExternalInput").ap()
    gamma_d = nc.dram_tensor("gamma", [D], f32, kind="ExternalInput").ap()
    beta_d = nc.dram_tensor("beta", [D], f32, kind="ExternalInput").ap()
    out_d = nc.dram_tensor("out", [S, D], f32, kind="ExternalOutput").ap()
    attn_d = nc.dram_tensor("attn", [S, S], f32, kind="ExternalOutput").ap()

    with tile.TileContext(nc) as tc, ExitStack() as ctx:
        consts = ctx.enter_context(tc.tile_pool(name="consts", bufs=1))
        wt_pool = ctx.enter_context(tc.tile_pool(name="wt", bufs=1))
        x_pool = ctx.enter_context(tc.tile_pool(name="xp", bufs=2))
        xt_pool = ctx.enter_context(tc.tile_pool(name="xt", bufs=1))
        qt_pool = ctx.enter_context(tc.tile_pool(name="qt", bufs=2))
        kt_pool = ctx.enter_context(tc.tile_pool(name="ktp", bufs=2))
        v_pool = ctx.enter_context(tc.tile_pool(name="vp", bufs=2))
        tail_pool = ctx.enter_context(tc.tile_pool(name="tails", bufs=3))
        out_pool = ctx.enter_context(tc.tile_pool(name="outp", bufs=2))
        attn_pool = ctx.enter_context(tc.tile_pool(name="attnp", bufs=3))
        piece_pool = ctx.enter_context(tc.tile_pool(name="piece", bufs=2))
        small = ctx.enter_context(tc.tile_pool(name="small", bufs=4))
        ps = ctx.enter_context(tc.tile_pool(name="psp", bufs=8, space="PSUM"))

        def psum():
            return ps.tile([P, 512], f32, tag="ps", name="ps")

        # ---- constants ----------------------------------------------------
        ident = consts.tile([P, P], f32, tag="ident", name="ident")
        nc.gpsimd.memset(ident[:], 0.0)
        nc.gpsimd.affine_select(
            out=ident[:], in_=ident[:], compare_op=mybir.AluOpType.not_equal,
            fill=1.0, base=0, pattern=[[-1, P]], channel_multiplier=1)

        # additive band mask over window coords: valid iff 0 <= jw - i <= 2k
        maskI = consts.tile([P, W_WIN], f32, tag="maskI", name="maskI")
        nc.gpsimd.memset(maskI[:], 0.0)
        nc.gpsimd.affine_select(  # jw - i >= 0
            out=maskI[:], in_=maskI[:], compare_op=mybir.AluOpType.is_ge,
            fill=NEG, base=0, pattern=[[1, W_WIN]], channel_multiplier=-1)
        nc.gpsimd.affine_select(  # 2k - (jw - i) >= 0
            out=maskI[:], in_=maskI[:], compare_op=mybir.AluOpType.is_ge,
            fill=NEG, base=2 * k, pattern=[[-1, W_WIN]], channel_multiplier=1)

        # biases striped per e-tile: (p, m) = b[m*128+p]; bq pre-scaled
        bq_sc = consts.tile([P, DT], f32, tag="bq", name="bq_sc")
        nc.sync.dma_start(bq_sc[:], bq_d.rearrange("(o p) -> p o", p=P))
        nc.scalar.mul(bq_sc[:], bq_sc[:], inv_scale)
        bk_sb = consts.tile([P, DT], f32, tag="bk", name="bk_sb")
        nc.sync.dma_start(bk_sb[:], bk_d.rearrange("(o p) -> p o", p=P))

        def bcast_load(vec_ap, name):
            t = consts.tile([P, D], f32, tag=name, name=name)
            src = bass.AP(tensor=vec_ap.tensor, offset=vec_ap.offset,
                          ap=[[0, P]] + list(vec_ap.ap))
            nc.gpsimd.dma_start(out=t[:], in_=src)
            return t

        bv_bc = bcast_load(bv_d, "bv_bc")
        gamma_bc = bcast_load(gamma_d, "gamma_bc") if apply_gb else None
        beta_bc = bcast_load(beta_d, "beta_bc") if apply_gb else None

        eps_t = consts.tile([P, 1], f32, tag="eps", name="eps_t")
        nc.vector.memset(eps_t[:], LN_EPS)

        # ---- weight transposes: WT[p, dt, e] = W[e, dt*128+p] -------------
        def load_wt(w_d, name):
            wt = wt_pool.tile([P, DT, D], f32, tag=name, name=name)
            for et in range(DT):
                wnat = x_pool.tile([P, TPC, D], f32, tag="x", name="wnat")
                nc.sync.dma_start(wnat[:, 0, :], w_d[et * P:(et + 1) * P, :])
                for dt in range(DT):
                    tp = psum()
                    nc.tensor.transpose(tp[:, :P], wnat[:, 0, dt * P:(dt + 1) * P],
                                        ident[:])
                    if dt % 2 == 0:
                        nc.scalar.copy(out=C(wt[:, dt, et * P:(et + 1) * P]),
                                       in_=tp[:, :P])
                    else:
                        nc.vector.tensor_copy(out=C(wt[:, dt, et * P:(et + 1) * P]),
                                              in_=tp[:, :P])
            return wt

        wqT = load_wt(wq_d, "wqT")
        wkT = load_wt(wk_d, "wkT")
        wvT = load_wt(wv_d, "wvT")

        # ---- main pipeline ------------------------------------------------
        xs, xts, qts, kts, vs, v_tails = {}, {}, {}, {}, {}, {}

        def b1(c):
            """Produce x/xT/QT/KT/V (+halo tails) for chunk c."""
            x_c = x_pool.tile([P, TPC, D], f32, tag="x", name="x_c")
            nc.sync.dma_start(
                x_c[:], x_d[c * CHUNK:(c + 1) * CHUNK, :]
                .rearrange("(u p) d -> p u d", p=P))
            xs[c] = x_c

            xt_c = xt_pool.tile([P, DT, CHUNK], f32, tag="xt", name="xt_c")
            for u in range(TPC):
                for dt in range(DT):
                    tp = psum()
                    nc.tensor.transpose(tp[:, :P], x_c[:, u, dt * P:(dt + 1) * P],
                                        ident[:])
                    if dt % 2 == 0:
                        nc.scalar.copy(out=C(xt_c[:, dt, u * P:(u + 1) * P]),
                                       in_=tp[:, :P])
                    else:
                        nc.vector.tensor_copy(
                            out=C(xt_c[:, dt, u * P:(u + 1) * P]),
                            in_=tp[:, :P])
            xts[c] = xt_c

            qt_c = qt_pool.tile([P, DT, CHUNK], f32, tag="qt", name="qt_c")
            # KT gets k halo columns on both sides so each score block is a
            # single contiguous rhs window: col h+j holds KT[:, c*CHUNK+j].
            kt_c = kt_pool.tile([P, DT, CHUNK + 2 * k], f32, tag="kt",
                                name="kt_c")
            if c == 0:  # never-written halos must still be finite
                nc.vector.memset(kt_c[:, :, 0:k], 0.0)
            if c == N_CHUNKS - 1:
                nc.vector.memset(kt_c[:, :, k + CHUNK:], 0.0)
            for wt, dst, off, bias, scale in (
                    (wqT, qt_c, 0, bq_sc, inv_scale),
                    (wkT, kt_c, k, bk_sb, 1.0)):
                for m in range(DT):
                    acc = psum()
                    for kk in range(DT):
                        nc.tensor.matmul(
                            acc[:, :CHUNK],
                            C(wt[:, kk, m * P:(m + 1) * P]),
                            C(xt_c[:, kk, :]),
                            start=(kk == 0), stop=(kk == DT - 1))
                    nc.scalar.activation(
                        C(dst[:, m, off:off + CHUNK]), acc[:, :CHUNK],
                        mybir.ActivationFunctionType.Identity,
                        bias=bias[:, m:m + 1], scale=scale)
            qts[c], kts[c] = qt_c, kt_c

            if c > 0:  # exchange halo columns with the previous chunk
                nc.vector.tensor_copy(
                    out=C(kts[c - 1][:, :, k + CHUNK:]),
                    in_=C(kt_c[:, :, k:2 * k]))
                nc.vector.tensor_copy(
                    out=C(kt_c[:, :, 0:k]),
                    in_=C(kts[c - 1][:, :, CHUNK:CHUNK + k]))

            v_c = v_pool.tile([P, TPC, D], f32, tag="v", name="v_c")
            for u in range(TPC):
                for nch in range(D // 512):
                    acc = psum()
                    for kk in range(DT):
                        nc.tensor.matmul(
                            acc[:],
                            C(xt_c[:, kk, u * P:(u + 1) * P]),
                            C(wvT[:, kk, nch * 512:(nch + 1) * 512]),
                            start=(kk == 0), stop=(kk == DT - 1))
                    nc.vector.tensor_tensor(
                        out=C(v_c[:, u, nch * 512:(nch + 1) * 512]), in0=acc[:],
                        in1=bv_bc[:, nch * 512:(nch + 1) * 512],
                        op=mybir.AluOpType.add)
            vs[c] = v_c

            # halo: last k V rows of each s-tile, re-based to partition 0
            # (matmul operands must start at partition 0/32/64) — packed in
            # the free dim: [:, 0:D] = tail of tile u=0, [:, D:2D] = u=1.
            vt = tail_pool.tile([k, 2 * D], f32, tag="vt", name="vt")
            nc.sync.dma_start(C(vt[:, 0:D]), C(v_c[P - k:P, 0, :]))
            nc.sync.dma_start(C(vt[:, D:2 * D]), C(v_c[P - k:P, 1, :]))
            v_tails[c] = vt

        def b2(c):
            """Scores, softmax, attn write, attn@V, residual+LN for chunk c.

            Phase-ordered: softmax for both s-tiles first, then attnV+LN for
            both — keeps ACT on one activation table (Exp..Exp, Sqrt..Sqrt)
            and overlaps tile 1's softmax with tile 0's attn@V on PE.
            """
            qt_c, kt_c, v_c, x_c = qts[c], kts[c], vs[c], xs[c]
            attns = {}
            for u in range(TPC):
                t = c * TPC + u
                lo = k if t == 0 else 0
                hi = P + k if t == N_TILES - 1 else W_WIN

                # one contiguous rhs window thanks to the KT halo columns
                s_ps = psum()
                for kk in range(DT):
                    nc.tensor.matmul(
                        s_ps[:, 0:W_WIN],
                        C(qt_c[:, kk, u * P:(u + 1) * P]),
                        C(kt_c[:, kk, u * P:u * P + W_WIN]),
                        start=(kk == 0), stop=(kk == DT - 1))

                attn_sb = attn_pool.tile([P, W_WIN], f32, tag="attn",
                                         name="attn_sb")
                nc.vector.tensor_tensor(out=attn_sb[:, lo:hi],
                                        in0=s_ps[:, lo:hi],
                                        in1=maskI[:, lo:hi],
                                        op=mybir.AluOpType.add)
                negmax = small.tile([P, 1], f32, tag="negmax", name="negmax")
                nc.vector.tensor_reduce(out=negmax[:], in_=attn_sb[:, lo:hi],
                                        axis=mybir.AxisListType.X,
                                        op=mybir.AluOpType.max, negate=True)
                rowsum = small.tile([P, 1], f32, tag="rowsum", name="rowsum")
                nc.scalar.activation(attn_sb[:, lo:hi], attn_sb[:, lo:hi],
                                     mybir.ActivationFunctionType.Exp,
                                     bias=negmax[:], scale=1.0,
                                     accum_out=rowsum[:])
                rinv = small.tile([P, 1], f32, tag="rinv", name="rinv")
                nc.vector.reciprocal(rinv[:], rowsum[:])
                nc.vector.tensor_scalar_mul(attn_sb[:, lo:hi],
                                            attn_sb[:, lo:hi], rinv[:])
                nc.sync.dma_start(
                    attn_d[t * P:(t + 1) * P, t * P - k + lo:t * P - k + hi],
                    attn_sb[:, lo:hi])
                attns[u] = attn_sb

            for u in range(TPC):
                t = c * TPC + u
                attn_sb = attns[u]
                # transposed attn pieces (lhsT for attn @ V) — all three share
                # one PSUM tile in disjoint column regions.
                tp = psum()
                pc_mid = piece_pool.tile([P, P], f32, tag="pmid", name="pc_mid")
                nc.tensor.transpose(tp[:, 0:P], attn_sb[:, k:P + k], ident[:])
                nc.vector.tensor_copy(out=C(pc_mid[:]), in_=tp[:, 0:P])
                pc_left = pc_right = None
                if t > 0:
                    pc_left = piece_pool.tile([k, P], f32, tag="pleft",
                                              name="pc_left")
                    nc.tensor.transpose(tp[:k, P:2 * P], attn_sb[:, 0:k],
                                        ident[:])
                    nc.vector.tensor_copy(out=C(pc_left[:]), in_=tp[:k, P:2 * P])
                if t < N_TILES - 1:
                    pc_right = piece_pool.tile([k, P], f32, tag="pright",
                                               name="pc_right")
                    nc.tensor.transpose(tp[:k, 2 * P:3 * P],
                                        attn_sb[:, P + k:W_WIN], ident[:])
                    nc.vector.tensor_copy(out=C(pc_right[:]),
                                          in_=tp[:k, 2 * P:3 * P])

                out_sb = out_pool.tile([P, D], f32, tag="out", name="out_sb")
                for nch in range(D // 512):
                    sl = slice(nch * 512, (nch + 1) * 512)
                    o_ps = psum()
                    mms = [(pc_mid[:], v_c[:, u, sl])]
                    if pc_left is not None:
                        left_rhs = (
                            v_tails[c - 1][:, D + nch * 512:D + (nch + 1) * 512]
                            if u == 0 else v_tails[c][:, sl])
                        mms.append((pc_left[:], left_rhs))
                    if pc_right is not None:
                        right_rhs = (v_c[0:k, 1, sl] if u == 0
                                     else vs[c + 1][0:k, 0, sl])
                        mms.append((pc_right[:], right_rhs))
                    for i, (lhsT, rhs) in enumerate(mms):
                        nc.tensor.matmul(o_ps[:], C(lhsT), C(rhs),
                                         start=(i == 0),
                                         stop=(i == len(mms) - 1))
                    nc.vector.tensor_tensor(out=out_sb[:, sl], in0=o_ps[:],
                                            in1=x_c[:, u, sl],
                                            op=mybir.AluOpType.add)

                # LayerNorm over D (free dim)
                stats = small.tile([P, 2, 6], f32, tag="stats", name="stats")
                nc.vector.bn_stats(stats[:, 0, :], out_sb[:, 0:512])
                nc.vector.bn_stats(stats[:, 1, :], out_sb[:, 512:1024])
                mv = small.tile([P, 2], f32, tag="mv", name="mv")
                nc.vector.bn_aggr(mv[:], stats[:])
                rstd = small.tile([P, 1], f32, tag="rstd", name="rstd")
                nc.scalar.activation(rstd[:], mv[:, 1:2],
                                     mybir.ActivationFunctionType.Sqrt,
                                     bias=eps_t[:])
                nc.vector.reciprocal(rstd[:], rstd[:])
                nc.vector.tensor_scalar(out_sb[:], out_sb[:],
                                        scalar1=mv[:, 0:1], scalar2=rstd[:],
                                        op0=mybir.AluOpType.subtract,
                                        op1=mybir.AluOpType.mult)
                if apply_gb:
                    nc.vector.tensor_tensor(out=out_sb[:], in0=out_sb[:],
                                            in1=gamma_bc[:],
                                            op=mybir.AluOpType.mult)
                    nc.vector.tensor_tensor(out=out_sb[:], in0=out_sb[:],
                                            in1=beta_bc[:],
                                            op=mybir.AluOpType.add)
                nc.sync.dma_start(out_d[t * P:(t + 1) * P, :], out_sb[:])

        b1(0)
        for c in range(1, N_CHUNKS):
            b1(c)
            b2(c - 1)
        b2(N_CHUNKS - 1)

    nc.compile()
    return nc


def _get_nc(k: int, apply_gb: bool = True):
    key = (k, MM_MODE, apply_gb)
    if key not in _BUILD_CACHE:
        _BUILD_CACHE[key] = _build(k, MM_MODE, apply_gb)
    return _BUILD_CACHE[key]


def run(x, Wq, bq, Wk, bk, Wv, bv, gamma, beta, k, trace=False):
    k = int(k)
    assert 1 <= k <= 32
    apply_gb = not (np.all(np.asarray(gamma) == 1.0)
                    and np.all(np.asarray(beta) == 0.0))
    nc = _get_nc(k, apply_gb)
    x = np.ascontiguousarray(np.asarray(x, dtype=np.float32))
    in_common = {
        "Wq": np.ascontiguousarray(np.asarray(Wq, np.float32)),
        "Wk": np.ascontiguousarray(np.asarray(Wk, np.float32)),
        "Wv": np.ascontiguousarray(np.asarray(Wv, np.float32)),
        "bq": np.ascontiguousarray(np.asarray(bq, np.float32)),
        "bk": np.ascontiguousarray(np.asarray(bk, np.float32)),
        "bv": np.ascontiguousarray(np.asarray(bv, np.float32)),
        "gamma": np.ascontiguousarray(np.asarray(gamma, np.float32)),
        "beta": np.ascontiguousarray(np.asarray(beta, np.float32)),
    }
    in_maps = [dict(in_common, x=x[b]) for b in range(B)]
    res = bass_utils.run_bass_kernel_spmd(
        nc, in_maps, core_ids=list(range(N_CORES)), trace=trace)
    out = np.stack([res.results[b]["out"] for b in range(B)])
    attn = np.stack([res.results[b]["attn"] for b in range(B)])
    return (out, attn), res


def kernel(x, Wq, bq, Wk, bk, Wv, bv, gamma, beta, k):
    (out, attn), _ = run(x, Wq, bq, Wk, bk, Wv, bv, gamma, beta, k)
    return out, attn


# revision 19
# speedup vs baseline: 1.2117x; 1.0377x over previous
"""Banded (sliding-window k=2) attention + residual + LayerNorm on 8 TRN2 cores.

Problem (per batch b): x (S=2048, D=1024)
  Q = x@Wq.T+bq ; K = x@Wk.T+bk ; V = x@Wv.T+bv
  scores = Q@K.T/sqrt(D), banded |i-j|<=k ; attn = softmax(scores)
  out = LN(attn@V + x) * gamma + beta
Returns (out (B,S,D), attn (B,S,S)).

Sharding: pure data-parallel over B — core b computes batch b. No collectives.

Per-core layout strategy (every matmul contracts over D, the contiguous axis
of both x and W, so both sides get PE-transposed on chip once):
  xT   [d,s]  <- PE transpose of x tiles (per 256-row chunk)
  WqT/WkT/WvT [d,e] <- PE transpose of weights (SBUF-resident)
  QT,KT [e,s] = WT.T @ xT ; V [s,e] = xT.T @ WvT (+bias on copy-out)
  score block per 128-row s-tile over window jw in [t*128-k, t*128+128+k):
      QT_tile.T @ KT[:, window] (+ additive band mask), softmax on-chip
  attn@V: PE-transpose attn-block pieces -> lhsT, rhs = V rows (+halo tiles)
  out = LN(attnV + x) via bn_stats/bn_aggr, * gamma + beta.
attn output: only band windows are written; the rest stays zero via the
pre-zeroed donated output buffers in run_bass_via_pjrt.
"""

from contextlib import ExitStack

import numpy as np

import concourse.bass as bass
import concourse.mybir as mybir
import concourse.tile as tile
from concourse import bacc, bass_utils

B, S, D = 8, 2048, 1024
N_CORES = 8
P = 128
CHUNK = 256                      # s rows per pipeline chunk (N for QT/KT matmuls)
N_CHUNKS = S // CHUNK            # 8
TPC = CHUNK // P                 # s-tiles per chunk = 2
N_TILES = S // P                 # 16
DT = D // P                      # d/e tiles = 8
LN_EPS = 1e-5
NEG = -1e30

MM_MODE = "f32r"                 # "f32r" (fast PE path) or "f32" (exact fp32)

_BUILD_CACHE: dict = {}


def _build(k: int, mm_mode: str, apply_gb: bool = True):
    W_WIN = P + 2 * k            # score window width per 128-row s-tile
    f32 = mybir.dt.float32
    inv_scale = 1.0 / float(np.sqrt(D))

    mm_dt = mybir.dt.float32r if mm_mode == "f32r" else f32

    def C(ap):
        if mm_mode == "f32r":
            return ap.bitcast(mybir.dt.float32r)
        return ap

    nc = bacc.Bacc(trn_type="TRN2", target_bir_lowering=False, debug=False,
                   num_devices=N_CORES, dynamic_dma_scratch_size=4096)

    x_d = nc.dram_tensor("x", [S, D], f32, kind="ExternalInput").ap()
    wq_d = nc.dram_tensor("Wq", [D, D], f32, kind="ExternalInput").ap()
    wk_d = nc.dram_tensor("Wk", [D, D], f32, kind="ExternalInput").ap()
    wv_d = nc.dram_tensor("Wv", [D, D], f32, kind="ExternalInput").ap()
    bq_d = nc.dram_tensor("bq", [D], f32, kind="ExternalInput").ap()
    bk_d = nc.dram_tensor("bk", [D], f32, kind="ExternalInput").ap()
    bv_d = nc.dram_tensor("bv", [D], f32, kind="ExternalInput").ap()
    gamma_d = nc.dram_tensor("gamma", [D], f32, kind="ExternalInput").ap()
    beta_d = nc.dram_tensor("beta", [D], f32, kind="ExternalInput").ap()
    out_d = nc.dram_tensor("out", [S, D], f32, kind="ExternalOutput").ap()
    attn_d = nc.dram_tensor("attn", [S, S], f32, kind="ExternalOutput").ap()

    with tile.TileContext(nc) as tc, ExitStack() as ctx:
        consts = ctx.enter_context(tc.tile_pool(name="consts", bufs=1))
        wt_pool = ctx.enter_context(tc.tile_pool(name="wt", bufs=1))
        x_pool = ctx.enter_context(tc.tile_pool(name="xp", bufs=2))
        xt_pool = ctx.enter_context(tc.tile_pool(name="xt", bufs=1))
        qt_pool = ctx.enter_context(tc.tile_pool(name="qt", bufs=2))
        kt_pool = ctx.enter_context(tc.tile_pool(name="ktp", bufs=2))
        v_pool = ctx.enter_context(tc.tile_pool(name="vp", bufs=2))
        tail_pool = ctx.enter_context(tc.tile_pool(name="tails", bufs=3))
        out_pool = ctx.enter_context(tc.tile_pool(name="outp", bufs=2))
        attn_pool = ctx.enter_context(tc.tile_pool(name="attnp", bufs=3))
        piece_pool = ctx.enter_context(tc.tile_pool(name="piece", bufs=2))
        small = ctx.enter_context(tc.tile_pool(name="small", bufs=4))
        ps = ctx.enter_context(tc.tile_pool(name="psp", bufs=8, space="PSUM"))

        def psum():
            return ps.tile([P, 512], f32, tag="ps", name="ps")

        # ---- constants ----------------------------------------------------
        ident0 = consts.tile([P, P], f32, tag="ident0", name="ident0")
        nc.gpsimd.memset(ident0[:], 0.0)
        nc.gpsimd.affine_select(
            out=ident0[:], in_=ident0[:],
            compare_op=mybir.AluOpType.not_equal,
            fill=1.0, base=0, pattern=[[-1, P]], channel_multiplier=1)
        # f32r-rounded copy — transposes consume the identity as a matmul
        # operand, and the BIR verifier wants an f32r-rounding producer.
        ident = consts.tile([P, P], f32, tag="ident", name="ident")
        nc.vector.tensor_copy(out=C(ident[:]), in_=ident0[:])

        # additive band mask over window coords: valid iff 0 <= jw - i <= 2k
        maskI = consts.tile([P, W_WIN], f32, tag="maskI", name="maskI")
        nc.gpsimd.memset(maskI[:], 0.0)
        nc.gpsimd.affine_select(  # jw - i >= 0
            out=maskI[:], in_=maskI[:], compare_op=mybir.AluOpType.is_ge,
            fill=NEG, base=0, pattern=[[1, W_WIN]], channel_multiplier=-1)
        nc.gpsimd.affine_select(  # 2k - (jw - i) >= 0
            out=maskI[:], in_=maskI[:], compare_op=mybir.AluOpType.is_ge,
            fill=NEG, base=2 * k, pattern=[[-1, W_WIN]], channel_multiplier=1)

        # biases striped per e-tile: (p, m) = b[m*128+p]; bq pre-scaled
        bq_sc = consts.tile([P, DT], f32, tag="bq", name="bq_sc")
        nc.sync.dma_start(bq_sc[:], bq_d.rearrange("(o p) -> p o", p=P))
        nc.scalar.mul(bq_sc[:], bq_sc[:], inv_scale)
        bk_sb = consts.tile([P, DT], f32, tag="bk", name="bk_sb")
        nc.sync.dma_start(bk_sb[:], bk_d.rearrange("(o p) -> p o", p=P))

        def bcast_load(vec_ap, name):
            t = consts.tile([P, D], f32, tag=name, name=name)
            src = bass.AP(tensor=vec_ap.tensor, offset=vec_ap.offset,
                          ap=[[0, P]] + list(vec_ap.ap))
            nc.gpsimd.dma_start(out=t[:], in_=src)
            return t

        bv_bc = bcast_load(bv_d, "bv_bc")
        gamma_bc = bcast_load(gamma_d, "gamma_bc") if apply_gb else None
        beta_bc = bcast_load(beta_d, "beta_bc") if apply_gb else None

        eps_t = consts.tile([P, 1], f32, tag="eps", name="eps_t")
        nc.vector.memset(eps_t[:], LN_EPS)

        # ---- weight transposes: WT[p, dt, e] = W[e, dt*128+p] -------------
        def load_wt(w_d, name):
            wt = wt_pool.tile([P, DT, D], f32, tag=name, name=name)
            for et in range(DT):
                wnat = x_pool.tile([P, TPC, D], f32, tag="x", name="wnat")
                nc.sync.dma_start(C(wnat[:, 0, :]),
                                  C(w_d[et * P:(et + 1) * P, :]))
                for dt in range(DT):
                    tp = psum()
                    nc.tensor.transpose(tp[:, :P].bitcast(mm_dt),
                                        C(wnat[:, 0, dt * P:(dt + 1) * P]),
                                        C(ident[:]))
                    if dt % 2 == 0:
                        nc.scalar.copy(out=C(wt[:, dt, et * P:(et + 1) * P]),
                                       in_=tp[:, :P])
                    else:
                        nc.vector.tensor_copy(out=C(wt[:, dt, et * P:(et + 1) * P]),
                                              in_=tp[:, :P])
            return wt

        wqT = load_wt(wq_d, "wqT")
        wkT = load_wt(wk_d, "wkT")
        wvT = load_wt(wv_d, "wvT")

        # ---- main pipeline ------------------------------------------------
        xs, xts, qts, kts, vs, v_tails = {}, {}, {}, {}, {}, {}

        def b1(c):
            """Produce x/xT/QT/KT/V (+halo tails) for chunk c."""
            x_c = x_pool.tile([P, TPC, D], f32, tag="x", name="x_c")
            nc.sync.dma_start(
                C(x_c[:]), C(x_d[c * CHUNK:(c + 1) * CHUNK, :]
                             .rearrange("(u p) d -> p u d", p=P)))
            xs[c] = x_c

            xt_c = xt_pool.tile([P, DT, CHUNK], f32, tag="xt", name="xt_c")
            for u in range(TPC):
                for dt in range(DT):
                    tp = psum()
                    nc.tensor.transpose(tp[:, :P].bitcast(mm_dt),
                                        C(x_c[:, u, dt * P:(dt + 1) * P]),
                                        C(ident[:]))
                    if dt % 2 == 0:
                        nc.scalar.copy(out=C(xt_c[:, dt, u * P:(u + 1) * P]),
                                       in_=tp[:, :P])
                    else:
                        nc.vector.tensor_copy(
                            out=C(xt_c[:, dt, u * P:(u + 1) * P]),
                            in_=tp[:, :P])
            xts[c] = xt_c

            qt_c = qt_pool.tile([P, DT, CHUNK], f32, tag="qt", name="qt_c")
            # KT gets k halo columns on both sides so each score block is a
            # single contiguous rhs window: col h+j holds KT[:, c*CHUNK+j].
            kt_c = kt_pool.tile([P, DT, CHUNK + 2 * k], f32, tag="kt",
                                name="kt_c")
            if c == 0:  # never-written halos must still be finite
                nc.vector.memset(kt_c[:, :, 0:k], 0.0)
            if c == N_CHUNKS - 1:
                nc.vector.memset(kt_c[:, :, k + CHUNK:], 0.0)
            for wt, dst, off, bias, scale in (
                    (wqT, qt_c, 0, bq_sc, inv_scale),
                    (wkT, kt_c, k, bk_sb, 1.0)):
                for m in range(DT):
                    acc = psum()
                    for kk in range(DT):
                        nc.tensor.matmul(
                            acc[:, :CHUNK],
                            C(wt[:, kk, m * P:(m + 1) * P]),
                            C(xt_c[:, kk, :]),
                            start=(kk == 0), stop=(kk == DT - 1))
                    nc.scalar.activation(
                        C(dst[:, m, off:off + CHUNK]), acc[:, :CHUNK],
                        mybir.ActivationFunctionType.Identity,
                        bias=bias[:, m:m + 1], scale=scale)
            qts[c], kts[c] = qt_c, kt_c

            if c > 0:  # exchange halo columns with the previous chunk
                nc.vector.tensor_copy(
                    out=C(kts[c - 1][:, :, k + CHUNK:]),
                    in_=C(kt_c[:, :, k:2 * k]))
                nc.vector.tensor_copy(
                    out=C(kt_c[:, :, 0:k]),
                    in_=C(kts[c - 1][:, :, CHUNK:CHUNK + k]))

            v_c = v_pool.tile([P, TPC, D], f32, tag="v", name="v_c")
            for u in range(TPC):
                for nch in range(D // 512):
                    acc = psum()
                    for kk in range(DT):
                        nc.tensor.matmul(
                            acc[:],
                            C(xt_c[:, kk, u * P:(u + 1) * P]),
                            C(wvT[:, kk, nch * 512:(nch + 1) * 512]),
                            start=(kk == 0), stop=(kk == DT - 1))
                    nc.vector.tensor_tensor(
                        out=C(v_c[:, u, nch * 512:(nch + 1) * 512]), in0=acc[:],
                        in1=bv_bc[:, nch * 512:(nch + 1) * 512],
                        op=mybir.AluOpType.add)
            vs[c] = v_c

            # halo: last k V rows of each s-tile, re-based to partition 0
            # (matmul operands must start at partition 0/32/64) — packed in
            # the free dim: [:, 0:D] = tail of tile u=0, [:, D:2D] = u=1.
            vt = tail_pool.tile([k, 2 * D], f32, tag="vt", name="vt")
            nc.sync.dma_start(C(vt[:, 0:D]), C(v_c[P - k:P, 0, :]))
            nc.sync.dma_start(C(vt[:, D:2 * D]), C(v_c[P - k:P, 1, :]))
            v_tails[c] = vt

        def b2(c):
            """Scores, softmax, attn write, attn@V, residual+LN for chunk c.

            Phase-ordered: softmax for both s-tiles first, then attnV+LN for
            both — keeps ACT on one activation table (Exp..Exp, Sqrt..Sqrt)
            and overlaps tile 1's softmax with tile 0's attn@V on PE.
            """
            qt_c, kt_c, v_c, x_c = qts[c], kts[c], vs[c], xs[c]
            attns = {}
            for u in range(TPC):
                t = c * TPC + u
                lo = k if t == 0 else 0
                hi = P + k if t == N_TILES - 1 else W_WIN

                # one contiguous rhs window thanks to the KT halo columns
                s_ps = psum()
                for kk in range(DT):
                    nc.tensor.matmul(
                        s_ps[:, 0:W_WIN],
                        C(qt_c[:, kk, u * P:(u + 1) * P]),
                        C(kt_c[:, kk, u * P:u * P + W_WIN]),
                        start=(kk == 0), stop=(kk == DT - 1))

                attn_sb = attn_pool.tile([P, W_WIN], f32, tag="attn",
                                         name="attn_sb")
                nc.vector.tensor_tensor(out=C(attn_sb[:, lo:hi]),
                                        in0=s_ps[:, lo:hi],
                                        in1=maskI[:, lo:hi],
                                        op=mybir.AluOpType.add)
                negmax = small.tile([P, 1], f32, tag="negmax", name="negmax")
                nc.vector.tensor_reduce(out=negmax[:], in_=attn_sb[:, lo:hi],
                                        axis=mybir.AxisListType.X,
                                        op=mybir.AluOpType.max, negate=True)
                rowsum = small.tile([P, 1], f32, tag="rowsum", name="rowsum")
                nc.scalar.activation(C(attn_sb[:, lo:hi]), attn_sb[:, lo:hi],
                                     mybir.ActivationFunctionType.Exp,
                                     bias=negmax[:], scale=1.0,
                                     accum_out=rowsum[:])
                rinv = small.tile([P, 1], f32, tag="rinv", name="rinv")
                nc.vector.reciprocal(rinv[:], rowsum[:])
                nc.vector.tensor_scalar_mul(C(attn_sb[:, lo:hi]),
                                            attn_sb[:, lo:hi], rinv[:])
                nc.sync.dma_start(
                    attn_d[t * P:(t + 1) * P, t * P - k + lo:t * P - k + hi],
                    attn_sb[:, lo:hi])
                attns[u] = attn_sb

            for u in range(TPC):
                t = c * TPC + u
                attn_sb = attns[u]
                # transposed attn pieces (lhsT for attn @ V) — all three share
                # one PSUM tile in disjoint column regions.
                tp = psum()
                pc_mid = piece_pool.tile([P, P], f32, tag="pmid", name="pc_mid")
                nc.tensor.transpose(tp[:, 0:P].bitcast(mm_dt),
                                    C(attn_sb[:, k:P + k]), C(ident[:]))
                nc.vector.tensor_copy(out=C(pc_mid[:]), in_=tp[:, 0:P])
                pc_left = pc_right = None
                if t > 0:
                    pc_left = piece_pool.tile([k, P], f32, tag="pleft",
                                              name="pc_left")
                    nc.tensor.transpose(tp[:k, P:2 * P].bitcast(mm_dt),
                                        C(attn_sb[:, 0:k]), C(ident[:]))
                    nc.vector.tensor_copy(out=C(pc_left[:]), in_=tp[:k, P:2 * P])
                if t < N_TILES - 1:
                    pc_right = piece_pool.tile([k, P], f32, tag="pright",
                                               name="pc_right")
                    nc.tensor.transpose(tp[:k, 2 * P:3 * P].bitcast(mm_dt),
                                        C(attn_sb[:, P + k:W_WIN]), C(ident[:]))
                    nc.vector.tensor_copy(out=C(pc_right[:]),
                                          in_=tp[:k, 2 * P:3 * P])

                out_sb = out_pool.tile([P, D], f32, tag="out", name="out_sb")
                for nch in range(D // 512):
                    sl = slice(nch * 512, (nch + 1) * 512)
                    o_ps = psum()
                    mms = [(pc_mid[:], v_c[:, u, sl])]
                    if pc_left is not None:
                        left_rhs = (
                            v_tails[c - 1][:, D + nch * 512:D + (nch + 1) * 512]
                            if u == 0 else v_tails[c][:, sl])
                        mms.append((pc_left[:], left_rhs))
                    if pc_right is not None:
                        right_rhs = (v_c[0:k, 1, sl] if u == 0
                                     else vs[c + 1][0:k, 0, sl])
                        mms.append((pc_right[:], right_rhs))
                    for i, (lhsT, rhs) in enumerate(mms):
                        nc.tensor.matmul(o_ps[:], C(lhsT), C(rhs),
                                         start=(i == 0),
                                         stop=(i == len(mms) - 1))
                    nc.vector.tensor_tensor(out=out_sb[:, sl], in0=o_ps[:],
                                            in1=x_c[:, u, sl],
                                            op=mybir.AluOpType.add)

                # LayerNorm over D (free dim)
                stats = small.tile([P, 2, 6], f32, tag="stats", name="stats")
                nc.vector.bn_stats(stats[:, 0, :], out_sb[:, 0:512])
                nc.vector.bn_stats(stats[:, 1, :], out_sb[:, 512:1024])
                mv = small.tile([P, 2], f32, tag="mv", name="mv")
                nc.vector.bn_aggr(mv[:], stats[:])
                rstd = small.tile([P, 1], f32, tag="rstd", name="rstd")
                nc.scalar.activation(rstd[:], mv[:, 1:2],
                                     mybir.ActivationFunctionType.Sqrt,
                                     bias=eps_t[:])
                nc.vector.reciprocal(rstd[:], rstd[:])
                nc.vector.tensor_scalar(out_sb[:], out_sb[:],
                                        scalar1=mv[:, 0:1], scalar2=rstd[:],
                                        op0=mybir.AluOpType.subtract,
                                        op1=mybir.AluOpType.mult)
                if apply_gb:
                    nc.vector.tensor_tensor(out=out_sb[:], in0=out_sb[:],
                                            in1=gamma_bc[:],
                                            op=mybir.AluOpType.mult)
                    nc.vector.tensor_tensor(out=out_sb[:], in0=out_sb[:],
                                            in1=beta_bc[:],
                                            op=mybir.AluOpType.add)
                nc.sync.dma_start(out_d[t * P:(t + 1) * P, :], out_sb[:])

        b1(0)
        for c in range(1, N_CHUNKS):
            b1(c)
            b2(c - 1)
        b2(N_CHUNKS - 1)

    nc.compile()
    return nc


def _get_nc(k: int, apply_gb: bool = True):
    key = (k, MM_MODE, apply_gb)
    if key not in _BUILD_CACHE:
        _BUILD_CACHE[key] = _build(k, MM_MODE, apply_gb)
    return _BUILD_CACHE[key]


def run(x, Wq, bq, Wk, bk, Wv, bv, gamma, beta, k, trace=False):
    k = int(k)
    assert 1 <= k <= 32
    apply_gb = not (np.all(np.asarray(gamma) == 1.0)
                    and np.all(np.asarray(beta) == 0.0))
    nc = _get_nc(k, apply_gb)
    x = np.ascontiguousarray(np.asarray(x, dtype=np.float32))
    in_common = {
        "Wq": np.ascontiguousarray(np.asarray(Wq, np.float32)),
        "Wk": np.ascontiguousarray(np.asarray(Wk, np.float32)),
        "Wv": np.ascontiguousarray(np.asarray(Wv, np.float32)),
        "bq": np.ascontiguousarray(np.asarray(bq, np.float32)),
        "bk": np.ascontiguousarray(np.asarray(bk, np.float32)),
        "bv": np.ascontiguousarray(np.asarray(bv, np.float32)),
        "gamma": np.ascontiguousarray(np.asarray(gamma, np.float32)),
        "beta": np.ascontiguousarray(np.asarray(beta, np.float32)),
    }
    in_maps = [dict(in_common, x=x[b]) for b in range(B)]
    res = bass_utils.run_bass_kernel_spmd(
        nc, in_maps, core_ids=list(range(N_CORES)), trace=trace)
    out = np.stack([res.results[b]["out"] for b in range(B)])
    attn = np.stack([res.results[b]["attn"] for b in range(B)])
    return (out, attn), res


def kernel(x, Wq, bq, Wk, bk, Wv, bv, gamma, beta, k):
    (out, attn), _ = run(x, Wq, bq, Wk, bk, Wv, bv, gamma, beta, k)
    return out, attn
